# revision 1
# baseline (speedup 1.0000x reference)
"""Trainium2 Bass kernel for nn_CABlock_26912265077025.

Architecture: CA-gating block (pools -> conv -> sigmoid gates -> x*gd*gh*gw)
followed by a 12000->4096->512->3 MLP and row L2-normalization.

Strategy: pure data parallelism over the batch across 8 NeuronCores
(512 rows each). On each core everything is expressed as matmuls with the
contraction dim on SBUF partitions:
  - the (dead-code-pruned) pools + conv = one [12032 x 50] matmul vs x^T
  - gate pre-activations = tiny selection matmuls
  - gates in log space: G = exp(-(Ssel^T @ softplus(-T))), applied to x in SBUF
  - mm1/mm2 in float32r (full-rate PE, ~1.5e-4 rel), mm3 + norm in float32.
Host side transposes/pads x and the weights so every DMA is contiguous.
"""

from contextlib import ExitStack

import numpy as np

import concourse.bass as bass
import concourse.mybir as mybir
import concourse.tile as tile
from concourse import bacc
from concourse.bass_utils import run_bass_kernel_spmd

N_CORES = 8
B_TOT = 4096
BS = B_TOT // N_CORES            # 512 batch rows per core
F = 12000                        # 3*10*10*40 flattened features
NK = 94                          # ceil(F/128)
FP = NK * 128                    # 12032 (rows F..FP-1 zero-padded)
H1, H2 = 4096, 512
NM1 = H1 // 128                  # 32 mm1 output tiles
NK2, NM2 = H1 // 128, H2 // 128  # 32, 4
KG = 6                           # w1 k-chunks per DMA group (paired m-tiles)
NKG = (NK + KG - 1) // KG        # 12 (last group has 6)
SELG = 4                         # ssel chunks per DMA group (24 groups, packed)
WPG = 8                          # wpool chunks per group (12 groups, last 6)
NWPG = (NK + WPG - 1) // WPG

f32 = mybir.dt.float32
f32r = mybir.dt.float32r
AF = mybir.ActivationFunctionType

_NC_CACHE = {}


def build_nc(KG=6, W1BUFS=2, Z1BUFS=2, PSMBUFS=2, SELBUFS=2, PSGBUFS=4, GSBUF=0, HOIST=0, ZG=4, DREP=1, RINGS=1, skip_gates=False, skip_pools=False, skip_mm2=False):
    nc = bacc.Bacc(None, target_bir_lowering=False)

    xt_d = nc.dram_tensor("xt", [NK, 128, BS], f32r, kind="ExternalInput")
    w1q_d = nc.dram_tensor("w1q", [NM1 // 2, 128, NK, 256], f32r, kind="ExternalInput")
    wpool_d = nc.dram_tensor("wpool", [NWPG, 128, WPG, 50], f32r, kind="ExternalInput")
    rmat_d = nc.dram_tensor("rmat", [50, 180], f32r, kind="ExternalInput")
    ssel_d = nc.dram_tensor("ssel", [(NK + SELG - 1) // SELG, 128, SELG, 256], f32r, kind="ExternalInput")
    w2k_d = nc.dram_tensor("w2k", [NK2, 128, NM2, 128], f32r, kind="ExternalInput")
    w3h_d = nc.dram_tensor("w3h", [128, NM2, 3], f32, kind="ExternalInput")
    b1_d = nc.dram_tensor("b1g", [128, NM1], f32, kind="ExternalInput")
    b2_d = nc.dram_tensor("b2g", [128, NM2], f32, kind="ExternalInput")
    b3_d = nc.dram_tensor("b3g", [3, 1], f32, kind="ExternalInput")
    out_d = nc.dram_tensor("out", [3, BS], f32, kind="ExternalOutput")

    nkg = (NK + KG - 1) // KG
    with tile.TileContext(nc) as tc, ExitStack() as ctx:
        consts = ctx.enter_context(tc.tile_pool(name="consts", bufs=1))
        dramp = ctx.enter_context(tc.tile_pool(name="dram", bufs=1, space="DRAM"))

        b1_sb = consts.tile([128, NM1], f32)
        b2_sb = consts.tile([128, NM2], f32)
        b3_sb = consts.tile([3, 1], f32)
        w3_sb = consts.tile([128, NM2, 3], f32)
        ones31 = consts.tile([3, 1], f32)
        ones13 = consts.tile([1, 3], f32)
        nc.sync.dma_start(b1_sb[:], b1_d[:])
        nc.sync.dma_start(b2_sb[:], b2_d[:])
        nc.sync.dma_start(b3_sb[:], b3_d[:])
        nc.sync.dma_start(w3_sb[:], w3h_d[:])
        nc.any.memset(ones31[:], 1.0)
        nc.any.memset(ones13[:], 1.0)

        zstack = ExitStack()
        zrem = NK % ZG
        zpool4 = zstack.enter_context(tc.tile_pool(name="z4", bufs=NK // ZG))
        zpool2 = zstack.enter_context(tc.tile_pool(name="z2", bufs=1 if zrem else 0)) if zrem else None
        z_slices = []

        def zsl(k):
            return z_slices[k]

        estack = ExitStack()
        psm2 = estack.enter_context(tc.tile_pool(name="psm2", bufs=1, space="PSUM"))

        gstack = ExitStack()
        gatep = gstack.enter_context(tc.tile_pool(name="gatep", bufs=1))

        dstack = ExitStack()
        if HOIST:
            w1p = dstack.enter_context(tc.tile_pool(name="w1p", bufs=W1BUFS))
            z1p = dstack.enter_context(tc.tile_pool(name="z1p", bufs=Z1BUFS))
            psm = dstack.enter_context(tc.tile_pool(name="psm", bufs=PSMBUFS, space="PSUM"))

        # ---------------- Phase A: x in (transposed) + pooled conv pre-acts
        with (
            tc.tile_pool(name="wpp", bufs=2) as wpp,
            tc.tile_pool(name="psy", bufs=1, space="PSUM") as psy,
        ):
            ypre = psy.tile([50, BS], f32)
            for g in range(NWPG):
                cnt = min(WPG, NK - g * WPG)
                wpt = wpp.tile([128, WPG, 50], f32r, tag="wp")
                nc.sync.dma_start(wpt[:, :cnt, :], wpool_d[g, :, :cnt, :])
                for j in range(0, cnt, ZG):
                    k = g * WPG + j
                    zn = min(ZG, NK - k)
                    zp = zpool4 if zn == ZG else zpool2
                    zt = zp.tile([128, zn, BS], f32r, tag=f"z{zn}")
                    nc.sync.dma_start(
                        zt[:], xt_d[k : k + zn].rearrange("k p b -> p k b")
                    )
                    for i in range(zn):
                        z_slices.append(zt[:, i, :])
                    if not (skip_pools or skip_gates):
                        for jj in range(zn):
                            nc.tensor.matmul(
                                ypre[:],
                                wpt[:, j + jj, :],
                                zsl(k + jj),
                                start=(k + jj == 0),
                                stop=(k + jj == NK - 1),
                            )

            # ------------- Phase B: y = relu(ypre); T = rmat^T @ y; Lp = softplus(-T)
            if skip_pools or skip_gates:
                Lpa = Lpb = None
            else:
              with (
                tc.tile_pool(name="yp", bufs=1) as yp,
                tc.tile_pool(name="pst", bufs=1, space="PSUM") as pst,
              ):
                y_sb = yp.tile([50, BS], f32r)
                rm_sb = yp.tile([50, 180], f32r)
                nc.sync.dma_start(rm_sb[:], rmat_d[:])
                nc.scalar.activation(y_sb[:], ypre[:], AF.Relu)
                Ta = pst.tile([128, BS], f32, tag="T")
                Tb = pst.tile([52, BS], f32, tag="T2")
                nc.tensor.matmul(Ta[:], rm_sb[:, 0:128], y_sb[:])
                nc.tensor.matmul(Tb[:], rm_sb[:, 128:180], y_sb[:])
                # softplus(-T) = ln(1 + exp(-T)) using only the ln/exp table
                Lpa = gatep.tile([128, BS], f32r)
                Lpb = gatep.tile([52, BS], f32r)
                nc.scalar.activation(Ta[:], Ta[:], AF.Exp, scale=-1.0)
                nc.scalar.activation(Tb[:], Tb[:], AF.Exp, scale=-1.0)
                nc.scalar.activation(Lpa[:], Ta[:], AF.Ln, bias=1.0)
                nc.scalar.activation(Lpb[:], Tb[:], AF.Ln, bias=1.0)

        # ---------------- Phase C: G = exp(-(Ssel^T @ Lp)); z = x * G (in place)
        with ExitStack() as cstack:
            sselp = cstack.enter_context(tc.tile_pool(name="sselp", bufs=SELBUFS))
            psg = cstack.enter_context(tc.tile_pool(name="psg", bufs=PSGBUFS, space="PSUM"))
            gsb = cstack.enter_context(tc.tile_pool(name="gsb", bufs=GSBUF)) if GSBUF else None
            for g in range(0 if skip_gates else (NK + SELG - 1) // SELG):
                sq = sselp.tile([128, SELG, 256], f32r, tag="sq")
                nc.sync.dma_start(sq[:], ssel_d[g])
                for j in range(min(SELG, NK - g * SELG)):
                    k = g * SELG + j
                    gp = psg.tile([128, BS], f32, tag="g")
                    nc.tensor.matmul(
                        gp[:], sq[:, j, 0:128], Lpa[:],
                        start=True, stop=False,
                    )
                    nc.tensor.matmul(
                        gp[:], sq[0:52, j, 128:256], Lpb[:],
                        start=False, stop=True,
                    )
                    if gsb is not None:
                        gt = gsb.tile([128, BS], f32, tag="gs")
                        nc.scalar.activation(gt[:], gp[:], AF.Exp, scale=-1.0)
                        nc.vector.tensor_mul(zsl(k), zsl(k), gt[:])
                    else:
                        nc.scalar.activation(gp[:], gp[:], AF.Exp, scale=-1.0)
                        nc.vector.tensor_mul(zsl(k), zsl(k), gp[:])

        # ---------------- Phase D: z1 = relu(w1 @ z + b1), spilled to DRAM
        # m-tiles processed in pairs: each w1 DMA feeds 16 matmuls
        if not HOIST:
            gstack.close()  # free Lp tiles before opening phase-D pools
            w1p = dstack.enter_context(tc.tile_pool(name="w1p", bufs=W1BUFS))
            z1p = dstack.enter_context(tc.tile_pool(name="z1p", bufs=Z1BUFS))
            psm = dstack.enter_context(tc.tile_pool(name="psm", bufs=PSMBUFS, space="PSUM"))
        w2p = dstack.enter_context(tc.tile_pool(name="w2p", bufs=1))
        acc2s = [
            psm2.tile([128, BS], f32, tag=f"mm2_{m2}", name=f"acc2_{m2}")
            for m2 in range(NM2)
        ]
        def emit_mm2(pending):
            for k2, z1t, w2t in pending:
                for m2 in range(NM2):
                    nc.tensor.matmul(
                        acc2s[m2][:],
                        w2t[:, m2, :],
                        z1t[:],
                        start=(k2 == 0),
                        stop=(k2 == NK2 - 1),
                        skip_group_check=True,
                    )
            pending.clear()

        if True:
          for _rep in range(DREP):
            pending = []
            for mt in range(0, NM1, 2):
                acc0 = psm.tile([128, BS], f32, tag="mm1a")
                acc1 = psm.tile([128, BS], f32, tag="mm1b")
                accs = (acc0, acc1)
                for g in range(nkg):
                    cnt = min(KG, NK - g * KG)
                    wtq = w1p.tile([128, KG, 256], f32r, tag="w1q")
                    qeng = nc.sync if (RINGS == 1 or g % 2 == 0) else (nc.gpsimd if RINGS == 3 else nc.scalar)
                    qeng.dma_start(
                        wtq[:, :cnt, :],
                        w1q_d[mt // 2, :, g * KG : g * KG + cnt, :],
                    )
                    wts = (wtq, wtq)
                    sls = (slice(0, 128), slice(128, 256))
                    for j in range(cnt):
                        k = g * KG + j
                        for mi in range(2):
                            nc.tensor.matmul(
                                accs[mi][:],
                                wts[mi][:, j, sls[mi]],
                                zsl(k),
                                start=(k == 0),
                                stop=(k == NK - 1),
                            )
                    if g == 1:
                        # deferred mm2 partials of the previous pair: by now
                        # their z1 evictions have drained off the ACT engine
                        emit_mm2(pending)
                for mi in range(2):
                    k2 = mt + mi
                    z1t = z1p.tile([128, BS], f32r, tag="z1")
                    nc.scalar.activation(
                        z1t[:], accs[mi][:], AF.Relu,
                        bias=b1_sb[:, k2 : k2 + 1],
                    )
                    w2t = w2p.tile([128, NM2, 128], f32r, tag="w2")
                    nc.sync.dma_start(w2t[:], w2k_d[k2])
                    pending.append((k2, z1t, w2t))
            emit_mm2(pending)

        dstack.close()
        gstack.close()  # no-op if already closed

        if skip_mm2:
            with tc.tile_pool(name="dummy", bufs=1) as dummy:
                d = dummy.tile([3, BS], f32)
                nc.any.memset(d[:], 0.0)
                nc.sync.dma_start(out_d[:], d[:])
            estack.close()
            zstack.close()
            nc.compile()
            return nc

        # ---------------- Phase E: z2 = relu(acc2 + b2); F: mm3 + normalize
        z2_tiles = []
        with (
            tc.tile_pool(name="z2p", bufs=NM2) as z2p,
            tc.tile_pool(name="tailp", bufs=1) as tailp,
            tc.tile_pool(name="psf", bufs=1, space="PSUM") as psf,
        ):
            for m2 in range(NM2):
                z2t = z2p.tile([128, BS], f32, tag="z2")
                nc.scalar.activation(
                    z2t[:], acc2s[m2][:], AF.Relu, bias=b2_sb[:, m2 : m2 + 1]
                )
                z2_tiles.append(z2t)

            # ------------- Phase F: z3 = w3 @ z2 + b3; out = z3 / max(||z3||, 1e-12)
            acc3 = psf.tile([3, BS], f32, tag="f")
            for k3 in range(NM2):
                nc.tensor.matmul(
                    acc3[:], w3_sb[:, k3, :], z2_tiles[k3][:],
                    start=(k3 == 0), stop=(k3 == NM2 - 1),
                )
            z3 = tailp.tile([3, BS], f32)
            nc.vector.tensor_scalar_add(z3[:], acc3[:], b3_sb[:])
            sq = tailp.tile([3, BS], f32)
            nc.scalar.activation(sq[:], z3[:], AF.Square)
            sps = psf.tile([1, BS], f32, tag="f2")
            nc.tensor.matmul(sps[:], ones31[:], sq[:])
            # 1/max(sqrt(s), 1e-12) = min(exp(-0.5*ln(s)), 1e12)
            lns = tailp.tile([1, BS], f32)
            nc.scalar.activation(lns[:], sps[:], AF.Ln)
            inv = tailp.tile([1, BS], f32)
            nc.scalar.activation(inv[:], lns[:], AF.Exp, scale=-0.5)
            nc.vector.tensor_scalar_min(inv[:], inv[:], 1e12)
            inv3 = psf.tile([3, BS], f32, tag="f3")
            nc.tensor.matmul(inv3[:], ones13[:], inv[:])
            outt = tailp.tile([3, BS], f32)
            nc.vector.tensor_mul(outt[:], z3[:], inv3[:])
            nc.sync.dma_start(out_d[:], outt[:])

        estack.close()
        zstack.close()

    nc.compile()
    return nc


def _prep_shared(conv_w, F_w, w1, b1, w2, b2, w3, b3):
    """Host-side weight layouts shared by all cores."""
    fa = np.arange(F)
    c_idx = fa // 4000
    d_idx = (fa // 400) % 10
    h_idx = (fa // 40) % 10
    w_idx = fa % 40

    # pooled conv: y_pre[j] = sum_f wp[f, j] * x^T[f, b]
    wp = np.zeros((NWPG * WPG * 128, 50), np.float32)
    wp[fa, h_idx] = conv_w[c_idx] / 400.0
    wp[fa, 10 + w_idx] = conv_w[c_idx] / 100.0
    wpool = np.ascontiguousarray(
        wp.reshape(NWPG, WPG, 128, 50).transpose(0, 2, 1, 3)
    )

    # rmat: T[r, b] = sum_j rmat[j, r] * y[j, b]
    rm = np.zeros((50, 180), np.float32)
    cc10 = np.repeat(np.arange(3), 10)
    rm[np.tile(np.arange(10), 3), np.arange(30)] = F_w[cc10]
    rm[np.tile(np.arange(10), 3), 30 + np.arange(30)] = F_w[cc10]
    cc40 = np.repeat(np.arange(3), 40)
    rm[10 + np.tile(np.arange(40), 3), 60 + np.arange(120)] = F_w[cc40]

    # selection: logG[f] = -(sel[f, :] @ Lp)
    sel = np.zeros((FP, 180), np.float32)
    sel[fa, c_idx * 10 + d_idx] = 1.0
    sel[fa, 30 + c_idx * 10 + h_idx] = 1.0
    sel[fa, 60 + c_idx * 40 + w_idx] = 1.0
    nsg = (NK + SELG - 1) // SELG
    selp = np.zeros((nsg * SELG * 128, 180), np.float32)
    selp[:FP] = sel
    selp = selp.reshape(nsg, SELG, 128, 180).transpose(0, 3, 1, 2)  # [g, r, j, p]
    ssel = np.zeros((nsg, 128, SELG, 256), np.float32)
    ssel[:, :, :, 0:128] = selp[:, 0:128]
    ssel[:, 0:52, :, 128:256] = selp[:, 128:180]
    ssel = np.ascontiguousarray(ssel)

    w1p_ = np.zeros((H1, FP), np.float32)
    w1p_[:, :F] = w1
    w1q = np.ascontiguousarray(
        w1p_.reshape(NM1 // 2, 2, 128, NK, 128).transpose(0, 4, 3, 1, 2)
    ).reshape(NM1 // 2, 128, NK, 256)
    w2k = np.ascontiguousarray(
        w2.reshape(NM2, 128, NK2, 128).transpose(2, 3, 0, 1)
    )
    w3h = np.ascontiguousarray(w3.reshape(3, NM2, 128).transpose(2, 1, 0))

    return {
        "wpool": wpool,
        "rmat": rm,
        "ssel": ssel,
        "w1q": w1q,
        "w2k": w2k,
        "w3h": w3h,
        "b1g": np.ascontiguousarray(b1.reshape(NM1, 128).T),
        "b2g": np.ascontiguousarray(b2.reshape(NM2, 128).T),
        "b3g": np.ascontiguousarray(b3.reshape(3, 1)),
    }


def make_in_maps(x, conv_w, F_w, w1, b1, w2, b2, w3, b3):
    x = np.asarray(x, np.float32).reshape(B_TOT, F)
    shared = _prep_shared(
        np.asarray(conv_w, np.float32).reshape(3),
        np.asarray(F_w, np.float32).reshape(3),
        np.asarray(w1, np.float32),
        np.asarray(b1, np.float32),
        np.asarray(w2, np.float32),
        np.asarray(b2, np.float32),
        np.asarray(w3, np.float32),
        np.asarray(b3, np.float32),
    )
    in_maps = []
    for c in range(N_CORES):
        xs = x[c * BS : (c + 1) * BS]
        xt = np.zeros((FP, BS), np.float32)
        xt[:F] = xs.T
        m = dict(shared)
        m["xt"] = xt.reshape(NK, 128, BS)
        in_maps.append(m)
    return in_maps


def get_nc():
    if "nc" not in _NC_CACHE:
        _NC_CACHE["nc"] = build_nc()
    return _NC_CACHE["nc"]


def kernel(**inputs) -> np.ndarray:
    nc = get_nc()
    in_maps = make_in_maps(**inputs)
    res = run_bass_kernel_spmd(nc, in_maps, core_ids=list(range(N_CORES)))
    out = np.concatenate([r["out"] for r in res.results], axis=1)  # [3, 4096]
    return np.ascontiguousarray(out.T, dtype=np.float32)



# revision 2
# speedup vs baseline: 1.7157x; 1.7157x over previous
"""Trainium2 Bass kernel for nn_CABlock_26912265077025 — fp8 DoubleRow version.

Architecture: CA-gating block (pools -> conv -> sigmoid gates -> x*gd*gh*gw)
followed by a 12000->4096->512->3 MLP and row L2-normalization.

Strategy: pure data parallelism over the batch across 8 NeuronCores
(512 rows each). The dominant 12000->4096 matmul runs in fp8e4m3 with
DoubleRow perf mode (2 k-tiles per instruction, 0.5 cyc/row) using a
3-term residual expansion for accuracy:
    w1@z ~= w8@z8 + w8r@z8 + w8@z8r
with w8 = fp8(w1*SW), w8r = fp8(w1*SW - w8), z8 = fp8(z*SZ),
z8r = fp8(z*SZ - z8). All four arrays share one effective scale so the
terms accumulate in a single PSUM group; the 1/(SW*SZ) descale folds
into the z1 relu eviction.

x streams in bf16 twice (pools pass + gating pass) so x, z8, z8r never
coexist in SBUF. Gates: ssel fp8 0/1 selection (SBUF-persistent) x
bf16 Lp matmuls; exp on ACT with bias=ln(SZ); z8 cast on GPSIMD;
residual via one DVE scalar_tensor_tensor. The first TRAIL m-tiles of
mm1 trail the gating loop (PSUM-limited), hiding most of phase C.
"""

from contextlib import ExitStack

import numpy as np
import ml_dtypes

import concourse.bass as bass
import concourse.mybir as mybir
import concourse.tile as tile
from concourse import bacc
from concourse.alu_op_type import AluOpType
from concourse.bass_utils import run_bass_kernel_spmd

N_CORES = 8
B_TOT = 4096
BS = B_TOT // N_CORES           # 512 batch rows per core
F = 12000                       # 3*10*10*40 flattened features
NK = 94                         # ceil(F/128) k-tiles
NP = NK // 2                    # 47 DoubleRow k-pairs
FP = NK * 128                   # 12032 (rows F..FP-1 zero-padded)
H1, H2 = 4096, 512
NM1 = H1 // 128                 # 32 mm1 output tiles
NK2, NM2 = H1 // 128, H2 // 128 # 32, 4
ZG = 4                          # k-tiles per z8/z8r pool tile (2 pairs)
XG = 8                          # k-tiles per phase-A x DMA group
SELG = 4                        # ssel k-tiles per group (baseline layout)
NSG = (NK + SELG - 1) // SELG   # 24
WPG = 8                         # wpool k-tiles per group
NWPG = (NK + WPG - 1) // WPG    # 12
TRAIL = 6                       # m-tiles trailing the gating loop
WIN = 8                         # k-pairs per w1 DMA window
NWIN = (NP + WIN - 1) // WIN    # 6 (last window has 7)
SW = 4096.0                     # w1 fp8 scale
SWP = 8192.0                    # wpool fp8 scale
SZ = 32.0                       # z fp8 scale

f32 = mybir.dt.float32
f32r = mybir.dt.float32r
bf16 = mybir.dt.bfloat16
f8 = mybir.dt.float8e4
AF = mybir.ActivationFunctionType
DR = mybir.MatmulPerfMode.DoubleRow
E4 = ml_dtypes.float8_e4m3
BF = ml_dtypes.bfloat16

_NC_CACHE = {}


def build_nc():
    nc = bacc.Bacc(None, target_bir_lowering=False)

    xt_d = nc.dram_tensor("xt", [NK, 128, BS], bf16, kind="ExternalInput")
    xt8_d = nc.dram_tensor("xt8", [NK, 128, BS], f8, kind="ExternalInput")
    w1q_d = nc.dram_tensor("w1q", [NM1, 2, 128, NP, 2, 128], f8, kind="ExternalInput")
    wpool_d = nc.dram_tensor("wpool", [NP, 128, 2, 64], f8, kind="ExternalInput")
    rmat_d = nc.dram_tensor("rmat", [50, 180], f32r, kind="ExternalInput")
    ssel_d = nc.dram_tensor("ssel", [NSG, 128, SELG, 256], f8, kind="ExternalInput")
    w2k_d = nc.dram_tensor("w2k", [NK2, 128, NM2, 128], bf16, kind="ExternalInput")
    w3h_d = nc.dram_tensor("w3h", [128, NM2, 3], f32r, kind="ExternalInput")
    b1_d = nc.dram_tensor("b1g", [128, NM1], f32, kind="ExternalInput")
    b2_d = nc.dram_tensor("b2g", [128, NM2], f32, kind="ExternalInput")
    b3_d = nc.dram_tensor("b3g", [3, 1], f32, kind="ExternalInput")
    out_d = nc.dram_tensor("out", [3, BS], f32, kind="ExternalOutput")

    with tile.TileContext(nc) as tc, ExitStack() as ctx:
        consts = ctx.enter_context(tc.tile_pool(name="consts", bufs=1))

        b1_sb = consts.tile([128, NM1], f32)
        b2_sb = consts.tile([128, NM2], f32)
        b3_sb = consts.tile([3, 1], f32)
        w3_sb = consts.tile([128, NM2, 3], f32r)
        ones31 = consts.tile([3, 1], f32)
        ones13 = consts.tile([1, 3], f32)
        lnsz = consts.tile([128, 1], f32)
        s1 = consts.tile([128, 1], f32)
        nc.any.memset(ones31[:], 1.0)
        nc.any.memset(ones13[:], 1.0)
        nc.any.memset(lnsz[:], float(np.log(SZ)))
        nc.any.memset(s1[:], float(1.0 / (SW * SZ)))

        # z8/z8r pools: persistent per-k-tile fp8 slices
        zstack = ExitStack()
        nzf, zrem = NK // ZG, NK % ZG
        z8p4 = zstack.enter_context(tc.tile_pool(name="z8p4", bufs=nzf))
        z8rp4 = zstack.enter_context(tc.tile_pool(name="z8rp4", bufs=nzf))
        z8p2 = zstack.enter_context(tc.tile_pool(name="z8p2", bufs=1)) if zrem else None
        z8rp2 = zstack.enter_context(tc.tile_pool(name="z8rp2", bufs=1)) if zrem else None
        z8_tiles, z8r_tiles = [], []

        def zsl(lst, k):
            return lst[k // ZG][:, k % ZG, :]

        def zpair(lst, p):
            k = 2 * p
            return lst[k // ZG][:, k % ZG : k % ZG + 2, :]

        # ---------------- Phase A: pooled conv pre-acts, fp8 DoubleRow
        with (
            tc.tile_pool(name="xa", bufs=3) as xa,
            tc.tile_pool(name="wpp", bufs=2) as wpp,
            tc.tile_pool(name="psy", bufs=1, space="PSUM") as psy,
        ):
            ypre = psy.tile([64, BS], f32)
            NXG = (NP + 3) // 4          # 12 groups of 4 pairs
            for gx in range(NXG):
                cnt = min(4, NP - gx * 4)
                wpt = wpp.tile([128, 4, 2, 64], f8, tag="wp")
                nc.scalar.dma_start(
                    wpt[:, :cnt], wpool_d[gx * 4 : gx * 4 + cnt].rearrange("g p i r -> p g i r")
                )
                xt_ = xa.tile([128, XG, BS], f8, tag="xa")
                nc.sync.dma_start(
                    xt_[:, : 2 * cnt, :],
                    xt8_d[gx * XG : gx * XG + 2 * cnt].rearrange("k p b -> p k b"),
                )
                for j in range(cnt):
                    pp = gx * 4 + j
                    nc.tensor.matmul(
                        ypre[:], wpt[:, j], xt_[:, 2 * j : 2 * j + 2, :],
                        start=(pp == 0), stop=(pp == NP - 1), perf_mode=DR,
                    )

            nc.scalar.dma_start(b1_sb[:], b1_d[:])
            nc.scalar.dma_start(b2_sb[:], b2_d[:])
            nc.scalar.dma_start(b3_sb[:], b3_d[:])
            nc.scalar.dma_start(w3_sb[:], w3h_d[:])

            # ------------- Phase B: y = relu(ypre); T = rmat^T@y; Lp = ln(1+exp(-T))
            with (
                tc.tile_pool(name="yp", bufs=1) as yp,
                tc.tile_pool(name="pst", bufs=1, space="PSUM") as pst,
            ):
                y_sb = yp.tile([50, BS], f32r)
                rm_sb = yp.tile([50, 180], f32r)
                swp = yp.tile([128, 1], f32)
                nc.any.memset(swp[:], float(1.0 / SWP))
                nc.scalar.dma_start(rm_sb[:], rmat_d[:])
                nc.scalar.activation(y_sb[:], ypre[0:50, :], AF.Relu, scale=swp[0:50])
                Ta = pst.tile([128, BS], f32, tag="T")
                Tb = pst.tile([52, BS], f32, tag="T2")
                nc.tensor.matmul(Ta[:], rm_sb[:, 0:128], y_sb[:])
                nc.tensor.matmul(Tb[:], rm_sb[:, 128:180], y_sb[:])
                Lpa = consts.tile([128, BS], bf16)
                Lpb = consts.tile([52, BS], bf16)
                nc.scalar.activation(Ta[:], Ta[:], AF.Exp, scale=-1.0)
                nc.scalar.activation(Tb[:], Tb[:], AF.Exp, scale=-1.0)
                nc.scalar.activation(Lpa[:], Ta[:], AF.Ln, bias=1.0)
                nc.scalar.activation(Lpb[:], Tb[:], AF.Ln, bias=1.0)

        # ---------------- Phase C + trailing mm1
        dstack = ExitStack()
        z1p = dstack.enter_context(tc.tile_pool(name="z1p", bufs=TRAIL + 2))
        w2p = dstack.enter_context(tc.tile_pool(name="w2p", bufs=TRAIL + 2))
        w1p = dstack.enter_context(tc.tile_pool(name="w1p", bufs=2))
        slabs = {}
        HS = 24  # pairs in first half-slab

        def dma_slab_half(m, half):
            lo, hi = (0, HS) if half == 0 else (HS, NP)
            if half == 0:
                w8b = w1p.tile([128, NP, 2, 128], f8, tag="w8", name=f"w8s{m}")
                w8rb = w1p.tile([128, NP, 2, 128], f8, tag="w8r", name=f"w8rs{m}")
                slabs[m] = (w8b, w8rb)
            w8b, w8rb = slabs[m]
            nc.sync.dma_start(w8b[:, lo:hi], w1q_d[m, 0, :, lo:hi])
            nc.sync.dma_start(w8rb[:, lo:hi], w1q_d[m, 1, :, lo:hi])

        tstack = ExitStack()
        w1tp = tstack.enter_context(tc.tile_pool(name="w1tp", bufs=2))
        pstrail = tstack.enter_context(tc.tile_pool(name="pstr", bufs=1, space="PSUM"))
        trail_acc = [pstrail.tile([128, BS], f32, tag=f"tr{m}", name=f"tr{m}") for m in range(TRAIL)]
        TWLENS = [3, 4, 4, 4, 4, 4, 4, 4, 4, 4, 4, 4]
        TWLO = [sum(TWLENS[:i]) for i in range(len(TWLENS))]
        assert sum(TWLENS) == NP
        trail_w = {}  # (term, win) -> tile [128, TRAIL, WIN, 2, 128]

        def dma_trail_windows(win):
            lo, cnt = TWLO[win], TWLENS[win]
            for t in range(2):
                wt = w1tp.tile([128, TRAIL, 4, 2, 128], f8, tag=f"tw{t}", name=f"tw{t}_{win}")
                nc.gpsimd.dma_start(
                    wt[:, :, :cnt],
                    w1q_d[0:TRAIL, t, :, lo : lo + cnt].rearrange("m k g i j -> k m g i j"),
                )
                trail_w[(t, win)] = wt

        with ExitStack() as cstack:
            xc = cstack.enter_context(tc.tile_pool(name="xc", bufs=4))
            sselp = cstack.enter_context(tc.tile_pool(name="sselp", bufs=3))
            psg = cstack.enter_context(tc.tile_pool(name="psg", bufs=2, space="PSUM"))
            gsp = cstack.enter_context(tc.tile_pool(name="gsp", bufs=3))
            tsp = cstack.enter_context(tc.tile_pool(name="tsp", bufs=3))

            dma_trail_windows(0)
            ssel_tiles = {}

            def issue_ssel(gs):
                st = sselp.tile([128, SELG, 256], f8, tag="ssel", name=f"ssel{gs}")
                nc.scalar.dma_start(st[:], ssel_d[gs])
                ssel_tiles[gs] = st

            issue_ssel(0)
            issue_ssel(1)

            def do_trail(p):
                win_, woff_ = 0, p
                for i, ln in enumerate(TWLENS):
                    if woff_ < ln:
                        win_ = i
                        break
                    woff_ -= ln
                z8p = zpair(z8_tiles, p)
                z8rp = zpair(z8r_tiles, p)
                w8w = trail_w[(0, win_)]
                w8rw = trail_w[(1, win_)]
                for m in range(TRAIL):
                    acc = trail_acc[m]
                    nc.tensor.matmul(acc[:], w8w[:, m, woff_], z8p, start=(p == 0),
                                     stop=False, perf_mode=DR, skip_group_check=True)
                    nc.tensor.matmul(acc[:], w8rw[:, m, woff_], z8p, start=False,
                                     stop=False, perf_mode=DR, skip_group_check=True)
                    nc.tensor.matmul(acc[:], w8w[:, m, woff_], z8rp, start=False,
                                     stop=(p == NP - 1), perf_mode=DR, skip_group_check=True)

            xq = []

            def issue_x(p):
                xct = xc.tile([128, 2, BS], bf16, tag="xc", name=f"xc{p}")
                nc.sync.dma_start(xct[:], xt_d[2 * p : 2 * p + 2].rearrange("k p b -> p k b"))
                xq.append(xct)

            for p0 in range(3):
                issue_x(p0)
            win = 0
            for p in range(NP):
                if p == TWLO[win] + TWLENS[win]:
                    win += 1
                woff = p - TWLO[win]
                if woff == 0 and win + 1 < len(TWLENS):
                    dma_trail_windows(win + 1)
                if p + 3 < NP:
                    issue_x(p + 3)
                if p == 39:
                    dma_slab_half(TRAIL, 0)
                if p == 44:
                    dma_slab_half(TRAIL, 1)
                xct = xq.pop(0)
                for kk in range(2):
                    k = 2 * p + kk
                    if k % ZG == 0:
                        zn = min(ZG, NK - k)
                        zp = z8p4 if zn == ZG else z8p2
                        zrp = z8rp4 if zn == ZG else z8rp2
                        z8_tiles.append(zp.tile([128, zn, BS], f8, tag=f"z8_{zn}", name=f"z8t{k}"))
                        z8r_tiles.append(zrp.tile([128, zn, BS], f8, tag=f"z8r_{zn}", name=f"z8rt{k}"))
                    gsel, jsel = k // SELG, k % SELG
                    if jsel == 0 and gsel + 2 < NSG:
                        issue_ssel(gsel + 2)
                    st = ssel_tiles[gsel]
                    gp = psg.tile([128, BS], f32, tag="gp")
                    nc.tensor.matmul(
                        gp[:], st[:, jsel, 0:128], Lpa[:],
                        start=True, stop=False,
                    )
                    nc.tensor.matmul(
                        gp[:], st[0:52, jsel, 128:256], Lpb[:],
                        start=False, stop=True,
                    )
                    g_sb = gsp.tile([128, BS], bf16, tag="g")
                    nc.scalar.activation(g_sb[:], gp[:], AF.Exp, scale=-1.0, bias=lnsz[:])
                    t_sb = tsp.tile([128, BS], bf16, tag="t")
                    nc.vector.tensor_mul(t_sb[:], xct[:, kk, :], g_sb[:])
                    nc.gpsimd.tensor_copy(zsl(z8_tiles, k), t_sb[:])
                    nc.vector.scalar_tensor_tensor(
                        zsl(z8r_tiles, k), zsl(z8_tiles, k), -1.0, t_sb[:],
                        AluOpType.mult, AluOpType.add,
                    )
                # trailing mm1 for m-tiles 0..TRAIL-1, lagging one pair
                if p > 0:
                    do_trail(p - 1)
            do_trail(NP - 1)

        # ---------------- Phase D: remaining mm1 m-tiles + interleaved mm2
        pending = []

        def emit_mm2():
            for k2, z1t, w2t in pending:
                for m2 in range(NM2):
                    nc.tensor.matmul(
                        acc2s[m2][:], w2t[:, m2, :], z1t[:],
                        start=(k2 == 0), stop=(k2 == NK2 - 1),
                        skip_group_check=True,
                    )
            pending.clear()

        def evict_z1(k2, acc):
            z1t = z1p.tile([128, BS], bf16, tag="z1")
            nc.scalar.activation(
                z1t[:], acc[:], AF.Relu, scale=s1[:], bias=b1_sb[:, k2 : k2 + 1]
            )
            w2t = w2p.tile([128, NM2, 128], bf16, tag="w2")
            nc.sync.dma_start(w2t[:], w2k_d[k2])
            pending.append((k2, z1t, w2t))

        for m in range(TRAIL):
            evict_z1(m, trail_acc[m])
        tstack.close()

        psm = dstack.enter_context(tc.tile_pool(name="psm", bufs=3, space="PSUM"))
        psm2 = dstack.enter_context(tc.tile_pool(name="psm2", bufs=1, space="PSUM"))
        acc2s = [psm2.tile([128, BS], f32, tag=f"mm2_{m2}", name=f"acc2_{m2}") for m2 in range(NM2)]

        for m in range(TRAIL, NM1):
            if m + 1 < NM1:
                dma_slab_half(m + 1, 0)
            acc = psm.tile([128, BS], f32, tag="mm1")
            w8b, w8rb = slabs.pop(m)
            for p in range(NP):
                if p == 20 and m + 1 < NM1:
                    dma_slab_half(m + 1, 1)
                nc.tensor.matmul(acc[:], w8b[:, p], zpair(z8_tiles, p), start=(p == 0),
                                 stop=False, perf_mode=DR, skip_group_check=True)
                nc.tensor.matmul(acc[:], w8rb[:, p], zpair(z8_tiles, p), start=False,
                                 stop=False, perf_mode=DR, skip_group_check=True)
                nc.tensor.matmul(acc[:], w8b[:, p], zpair(z8r_tiles, p), start=False,
                                 stop=(p == NP - 1), perf_mode=DR, skip_group_check=True)
                if p == 8:
                    emit_mm2()
            evict_z1(m, acc)
        emit_mm2()

        dstack.close()

        # ---------------- Phase E/F: z2 = relu(acc2 + b2); mm3 + normalize
        with (
            tc.tile_pool(name="z2p", bufs=NM2) as z2p,
            tc.tile_pool(name="tailp", bufs=1) as tailp,
            tc.tile_pool(name="psf", bufs=1, space="PSUM") as psf,
        ):
            z2_tiles = []
            for m2 in range(NM2):
                z2t = z2p.tile([128, BS], f32r, tag="z2")
                nc.scalar.activation(
                    z2t[:], acc2s[m2][:], AF.Relu, bias=b2_sb[:, m2 : m2 + 1]
                )
                z2_tiles.append(z2t)

            acc3 = psf.tile([3, BS], f32, tag="f")
            for k3 in range(NM2):
                nc.tensor.matmul(
                    acc3[:], w3_sb[:, k3, :], z2_tiles[k3][:],
                    start=(k3 == 0), stop=(k3 == NM2 - 1),
                )
            z3 = tailp.tile([3, BS], f32)
            nc.vector.tensor_scalar_add(z3[:], acc3[:], b3_sb[:])
            sq = tailp.tile([3, BS], f32)
            nc.scalar.activation(sq[:], z3[:], AF.Square)
            sps = psf.tile([1, BS], f32, tag="f2")
            nc.tensor.matmul(sps[:], ones31[:], sq[:])
            lns = tailp.tile([1, BS], f32)
            nc.scalar.activation(lns[:], sps[:], AF.Ln)
            inv = tailp.tile([1, BS], f32)
            nc.scalar.activation(inv[:], lns[:], AF.Exp, scale=-0.5)
            nc.vector.tensor_scalar_min(inv[:], inv[:], 1e12)
            inv3 = psf.tile([3, BS], f32, tag="f3")
            nc.tensor.matmul(inv3[:], ones13[:], inv[:])
            outt = tailp.tile([3, BS], f32)
            nc.vector.tensor_mul(outt[:], z3[:], inv3[:])
            nc.sync.dma_start(out_d[:], outt[:])

        zstack.close()

    nc.compile()
    return nc


def _prep_shared(conv_w, F_w, w1, b1, w2, b2, w3, b3):
    """Host-side weight layouts shared by all cores."""
    fa = np.arange(F)
    c_idx = fa // 4000
    d_idx = (fa // 400) % 10
    h_idx = (fa // 40) % 10
    w_idx = fa % 40

    # pooled conv: y_pre[j] = sum_f wp[f, j] * x^T[f, b], fp8*SWP pair blocks
    wp = np.zeros((FP, 50), np.float32)
    wp[fa, h_idx] = conv_w[c_idx] * (SWP / 400.0)
    wp[fa, 10 + w_idx] = conv_w[c_idx] * (SWP / 100.0)
    wp64 = np.zeros((FP, 64), np.float32)
    wp64[:, :50] = wp
    wpool = np.ascontiguousarray(
        np.clip(wp64, -240, 240).astype(E4).reshape(NP, 2, 128, 64).transpose(0, 2, 1, 3)
    )

    # rmat: T[r, b] = sum_j rmat[j, r] * y[j, b]
    rm = np.zeros((50, 180), np.float32)
    cc10 = np.repeat(np.arange(3), 10)
    rm[np.tile(np.arange(10), 3), np.arange(30)] = F_w[cc10]
    rm[np.tile(np.arange(10), 3), 30 + np.arange(30)] = F_w[cc10]
    cc40 = np.repeat(np.arange(3), 40)
    rm[10 + np.tile(np.arange(40), 3), 60 + np.arange(120)] = F_w[cc40]

    # selection: gate pre-act[f] = sel[f, :] @ Lp  (0/1 entries, exact in fp8)
    sel = np.zeros((FP, 180), np.float32)
    sel[fa, c_idx * 10 + d_idx] = 1.0
    sel[fa, 30 + c_idx * 10 + h_idx] = 1.0
    sel[fa, 60 + c_idx * 40 + w_idx] = 1.0
    selp = np.zeros((NSG * SELG * 128, 180), np.float32)
    selp[:FP] = sel
    selp = selp.reshape(NSG, SELG, 128, 180).transpose(0, 3, 1, 2)  # [g, r, j, p]
    ssel = np.zeros((NSG, 128, SELG, 256), np.float32)
    ssel[:, :, :, 0:128] = selp[:, 0:128]
    ssel[:, 0:52, :, 128:256] = selp[:, 128:180]
    ssel = np.ascontiguousarray(ssel).astype(E4)

    # w1 fp8 pair: w8 = fp8(w1*SW), w8r = fp8(w1*SW - w8)
    w1p_ = np.zeros((H1, FP), np.float32)
    w1p_[:, :F] = w1 * SW
    w8 = np.clip(w1p_, -240, 240).astype(E4)
    w8r = np.clip(w1p_ - w8.astype(np.float32), -240, 240).astype(E4)
    # [H1, FP] -> [m, M, g, i, K] -> [m, K, g, i, M]
    def pack(a):
        return np.ascontiguousarray(
            a.reshape(NM1, 128, NP, 2, 128).transpose(0, 4, 2, 3, 1)
        )
    w1q = np.stack([pack(w8), pack(w8r)], axis=1)  # [m, t, K, g, i, M]

    w2k = np.ascontiguousarray(
        w2.reshape(NM2, 128, NK2, 128).transpose(2, 3, 0, 1)
    ).astype(BF)
    w3h = np.ascontiguousarray(w3.reshape(3, NM2, 128).transpose(2, 1, 0))

    return {
        "wpool": wpool,
        "rmat": rm,
        "ssel": ssel,
        "w1q": w1q,
        "w2k": w2k,
        "w3h": w3h,
        "b1g": np.ascontiguousarray(b1.reshape(NM1, 128).T),
        "b2g": np.ascontiguousarray(b2.reshape(NM2, 128).T),
        "b3g": np.ascontiguousarray(b3.reshape(3, 1)),
    }


def make_in_maps(x, conv_w, F_w, w1, b1, w2, b2, w3, b3):
    x = np.asarray(x, np.float32).reshape(B_TOT, F)
    shared = _prep_shared(
        np.asarray(conv_w, np.float32).reshape(3),
        np.asarray(F_w, np.float32).reshape(3),
        np.asarray(w1, np.float32),
        np.asarray(b1, np.float32),
        np.asarray(w2, np.float32),
        np.asarray(b2, np.float32),
        np.asarray(w3, np.float32),
        np.asarray(b3, np.float32),
    )
    in_maps = []
    for c in range(N_CORES):
        xs = x[c * BS : (c + 1) * BS]
        xt = np.zeros((FP, BS), np.float32)
        xt[:F] = xs.T
        m = dict(shared)
        m["xt"] = xt.reshape(NK, 128, BS).astype(BF)
        m["xt8"] = np.clip(xt.reshape(NK, 128, BS), -240, 240).astype(E4)
        in_maps.append(m)
    return in_maps


def get_nc():
    if "nc" not in _NC_CACHE:
        _NC_CACHE["nc"] = build_nc()
    return _NC_CACHE["nc"]


def kernel(**inputs) -> np.ndarray:
    nc = get_nc()
    in_maps = make_in_maps(**inputs)
    res = run_bass_kernel_spmd(nc, in_maps, core_ids=list(range(N_CORES)))
    out = np.concatenate([r["out"] for r in res.results], axis=1)  # [3, 4096]
    return np.ascontiguousarray(out.T, dtype=np.float32)


# revision 4
# speedup vs baseline: 1.7324x; 1.0097x over previous
"""Trainium2 Bass kernel for nn_CABlock_26912265077025 — fp8 DoubleRow version.

Architecture: CA-gating block (pools -> conv -> sigmoid gates -> x*gd*gh*gw)
followed by a 12000->4096->512->3 MLP and row L2-normalization.

Strategy: pure data parallelism over the batch across 8 NeuronCores
(512 rows each). The dominant 12000->4096 matmul runs in fp8e4m3 with
DoubleRow perf mode (2 k-tiles per instruction, 0.5 cyc/row) using a
3-term residual expansion for accuracy:
    w1@z ~= w8@z8 + w8r@z8 + w8@z8r
with w8 = fp8(w1*SW), w8r = fp8(w1*SW - w8), z8 = fp8(z*SZ),
z8r = fp8(z*SZ - z8). All four arrays share one effective scale so the
terms accumulate in a single PSUM group; the 1/(SW*SZ) descale folds
into the z1 relu eviction.

x streams in bf16 twice (pools pass + gating pass) so x, z8, z8r never
coexist in SBUF. Gates: ssel fp8 0/1 selection (SBUF-persistent) x
bf16 Lp matmuls; exp on ACT with bias=ln(SZ); z8 cast on GPSIMD;
residual via one DVE scalar_tensor_tensor. The first TRAIL m-tiles of
mm1 trail the gating loop (PSUM-limited), hiding most of phase C.
"""

from contextlib import ExitStack

import numpy as np
import ml_dtypes

import concourse.bass as bass
import concourse.mybir as mybir
import concourse.tile as tile
from concourse import bacc
from concourse.alu_op_type import AluOpType
from concourse.bass_utils import run_bass_kernel_spmd

N_CORES = 8
B_TOT = 4096
BS = B_TOT // N_CORES           # 512 batch rows per core
F = 12000                       # 3*10*10*40 flattened features
NK = 94                         # ceil(F/128) k-tiles
NP = NK // 2                    # 47 DoubleRow k-pairs
FP = NK * 128                   # 12032 (rows F..FP-1 zero-padded)
H1, H2 = 4096, 512
NM1 = H1 // 128                 # 32 mm1 output tiles
NK2, NM2 = H1 // 128, H2 // 128 # 32, 4
ZG = 4                          # k-tiles per z8/z8r pool tile (2 pairs)
XG = 8                          # k-tiles per phase-A x DMA group
SELG = 4                        # ssel k-tiles per group (baseline layout)
NSG = (NK + SELG - 1) // SELG   # 24
WPG = 8                         # wpool k-tiles per group
NWPG = (NK + WPG - 1) // WPG    # 12
TRAIL = 6                       # m-tiles trailing the gating loop
WIN = 8                         # k-pairs per w1 DMA window
NWIN = (NP + WIN - 1) // WIN    # 6 (last window has 7)
SW = 4096.0                     # w1 fp8 scale
SWP = 8192.0                    # wpool fp8 scale
SZ = 32.0                       # z fp8 scale

f32 = mybir.dt.float32
f32r = mybir.dt.float32r
bf16 = mybir.dt.bfloat16
f8 = mybir.dt.float8e4
AF = mybir.ActivationFunctionType
DR = mybir.MatmulPerfMode.DoubleRow
E4 = ml_dtypes.float8_e4m3
BF = ml_dtypes.bfloat16

_NC_CACHE = {}


def build_nc():
    nc = bacc.Bacc(None, target_bir_lowering=False)

    xt_d = nc.dram_tensor("xt", [NK, 128, BS], bf16, kind="ExternalInput")
    xt8_d = nc.dram_tensor("xt8", [NK, 128, BS], f8, kind="ExternalInput")
    w1q_d = nc.dram_tensor("w1q", [NM1, 2, 128, NP, 2, 128], f8, kind="ExternalInput")
    wpool_d = nc.dram_tensor("wpool", [NP, 128, 2, 64], f8, kind="ExternalInput")
    rmat_d = nc.dram_tensor("rmat", [50, 180], f32r, kind="ExternalInput")
    ssel_d = nc.dram_tensor("ssel", [NSG, 128, SELG, 256], f8, kind="ExternalInput")
    w2k_d = nc.dram_tensor("w2k", [NK2, 128, NM2, 128], bf16, kind="ExternalInput")
    w3h_d = nc.dram_tensor("w3h", [128, NM2, 3], f32r, kind="ExternalInput")
    b1_d = nc.dram_tensor("b1g", [128, NM1], f32, kind="ExternalInput")
    b2_d = nc.dram_tensor("b2g", [128, NM2], f32, kind="ExternalInput")
    b3_d = nc.dram_tensor("b3g", [3, 1], f32, kind="ExternalInput")
    ones_d = nc.dram_tensor("onesg", [3, 3], f32r, kind="ExternalInput")
    out_d = nc.dram_tensor("out", [3, BS], f32, kind="ExternalOutput")

    with tile.TileContext(nc) as tc, ExitStack() as ctx:
        consts = ctx.enter_context(tc.tile_pool(name="consts", bufs=1))

        b1_sb = consts.tile([128, NM1], f32)
        b2_sb = consts.tile([128, NM2], f32)
        b3_sb = consts.tile([3, 1], f32)
        w3_sb = consts.tile([128, NM2, 3], f32r)
        ones31 = consts.tile([3, 1], f32r)
        ones13 = consts.tile([1, 3], f32r)
        lnsz = consts.tile([128, 1], f32)
        s1 = consts.tile([128, 1], f32)
        nc.scalar.dma_start(ones31[:], ones_d[:, 0:1])
        nc.scalar.dma_start(ones13[:], ones_d[0:1, :])
        nc.any.memset(lnsz[:], float(np.log(SZ)))
        nc.any.memset(s1[:], float(1.0 / (SW * SZ)))

        # z8/z8r pools: persistent per-k-tile fp8 slices
        zstack = ExitStack()
        nzf, zrem = NK // ZG, NK % ZG
        z8p4 = zstack.enter_context(tc.tile_pool(name="z8p4", bufs=nzf))
        z8rp4 = zstack.enter_context(tc.tile_pool(name="z8rp4", bufs=nzf))
        z8p2 = zstack.enter_context(tc.tile_pool(name="z8p2", bufs=1)) if zrem else None
        z8rp2 = zstack.enter_context(tc.tile_pool(name="z8rp2", bufs=1)) if zrem else None
        z8_tiles, z8r_tiles = [], []

        def zsl(lst, k):
            return lst[k // ZG][:, k % ZG, :]

        def zpair(lst, p):
            k = 2 * p
            return lst[k // ZG][:, k % ZG : k % ZG + 2, :]

        # pools/issuance opened early so phase-C lead-in DMAs overlap phase A
        dstack = ExitStack()
        w1tp = dstack.enter_context(tc.tile_pool(name="w1tp", bufs=2))
        slabs = {}
        HS = 24  # pairs in first half-slab

        def dma_slab_half(m, half):
            lo, hi = (0, HS) if half == 0 else (HS, NP)
            w8h = w1p.tile([128, HS, 2, 128], f8, tag="w8", name=f"w8s{m}_{half}")
            w8rh = w1p.tile([128, HS, 2, 128], f8, tag="w8r", name=f"w8rs{m}_{half}")
            nc.sync.dma_start(w8h[:, : hi - lo], w1q_d[m, 0, :, lo:hi])
            nc.sync.dma_start(w8rh[:, : hi - lo], w1q_d[m, 1, :, lo:hi])
            slabs[(m, half)] = (w8h, w8rh)

        TWLENS = [3, 4, 4, 4, 4, 4, 4, 4, 4, 4, 4, 4]
        TWLO = [sum(TWLENS[:i]) for i in range(len(TWLENS))]
        assert sum(TWLENS) == NP
        trail_w = {}  # (term, win) -> tile [128, TRAIL, 4, 2, 128]

        def dma_trail_windows(win):
            lo, cnt = TWLO[win], TWLENS[win]
            for t in range(2):
                wt = w1tp.tile([128, TRAIL, 4, 2, 128], f8, tag=f"tw{t}", name=f"tw{t}_{win}")
                nc.gpsimd.dma_start(
                    wt[:, :, :cnt],
                    w1q_d[0:TRAIL, t, :, lo : lo + cnt].rearrange("m k g i j -> k m g i j"),
                )
                trail_w[(t, win)] = wt

        xc = dstack.enter_context(tc.tile_pool(name="xc", bufs=3))
        sselp = dstack.enter_context(tc.tile_pool(name="sselp", bufs=3))
        ssel_tiles = {}

        def issue_ssel(gs):
            st = sselp.tile([128, SELG, 256], f8, tag="ssel", name=f"ssel{gs}")
            nc.scalar.dma_start(st[:], ssel_d[gs])
            ssel_tiles[gs] = st

        xq = []

        def issue_x(p):
            xct = xc.tile([128, 2, BS], bf16, tag="xc", name=f"xc{p}")
            nc.sync.dma_start(xct[:], xt_d[2 * p : 2 * p + 2].rearrange("k p b -> p k b"))
            xq.append(xct)

        # ---------------- Phase A: pooled conv pre-acts, fp8 DoubleRow
        with (
            tc.tile_pool(name="xa", bufs=2) as xa,
            tc.tile_pool(name="wpp", bufs=2) as wpp,
            tc.tile_pool(name="psy", bufs=1, space="PSUM") as psy,
        ):
            ypre = psy.tile([64, BS], f32)
            ACH = [12, 12, 12, 11]       # pairs per x8 chunk
            for gx in range(4):
                pcnt = ACH[gx]
                plo = sum(ACH[:gx])
                wpt = wpp.tile([128, 12, 2, 64], f8, tag="wp")
                nc.scalar.dma_start(
                    wpt[:, :pcnt], wpool_d[plo : plo + pcnt].rearrange("g p i r -> p g i r")
                )
                xt_ = xa.tile([128, 24, BS], f8, tag="xa")
                nc.sync.dma_start(
                    xt_[:, : 2 * pcnt, :],
                    xt8_d[2 * plo : 2 * plo + 2 * pcnt].rearrange("k p b -> p k b"),
                )
                for j in range(pcnt):
                    pp = plo + j
                    nc.tensor.matmul(
                        ypre[:], wpt[:, j], xt_[:, 2 * j : 2 * j + 2, :],
                        start=(pp == 0), stop=(pp == NP - 1), perf_mode=DR,
                    )
                if gx == 0:
                    dma_trail_windows(0)
                elif gx == 2:
                    issue_ssel(0)
                elif gx == 3:
                    issue_ssel(1)
                    for p0 in range(2):
                        issue_x(p0)

            nc.scalar.dma_start(b1_sb[:], b1_d[:])
            nc.scalar.dma_start(b2_sb[:], b2_d[:])
            nc.scalar.dma_start(b3_sb[:], b3_d[:])
            nc.scalar.dma_start(w3_sb[:], w3h_d[:])

            # ------------- Phase B: y = relu(ypre); T = rmat^T@y; Lp = ln(1+exp(-T))
            with (
                tc.tile_pool(name="yp", bufs=1) as yp,
                tc.tile_pool(name="pst", bufs=1, space="PSUM") as pst,
            ):
                y_sb = yp.tile([50, BS], f32r)
                rm_sb = yp.tile([50, 180], f32r)
                swp = yp.tile([128, 1], f32)
                nc.any.memset(swp[:], float(1.0 / SWP))
                nc.scalar.dma_start(rm_sb[:], rmat_d[:])
                nc.scalar.activation(y_sb[:], ypre[0:50, :], AF.Relu, scale=swp[0:50])
                Ta = pst.tile([128, BS], f32, tag="T")
                Tb = pst.tile([52, BS], f32, tag="T2")
                nc.tensor.matmul(Ta[:], rm_sb[:, 0:128], y_sb[:])
                nc.tensor.matmul(Tb[:], rm_sb[:, 128:180], y_sb[:])
                Lpa = consts.tile([128, BS], bf16)
                Lpb = consts.tile([52, BS], bf16)
                nc.scalar.activation(Ta[:], Ta[:], AF.Exp, scale=-1.0)
                nc.scalar.activation(Tb[:], Tb[:], AF.Exp, scale=-1.0)
                nc.scalar.activation(Lpa[:], Ta[:], AF.Ln, bias=1.0)
                nc.scalar.activation(Lpb[:], Tb[:], AF.Ln, bias=1.0)

        # ---------------- Phase C + trailing mm1
        z1p = dstack.enter_context(tc.tile_pool(name="z1p", bufs=TRAIL + 1))
        w2p = dstack.enter_context(tc.tile_pool(name="w2p", bufs=TRAIL + 1))
        w1p = dstack.enter_context(tc.tile_pool(name="w1p", bufs=3))
        tstack = ExitStack()
        gsp = tstack.enter_context(tc.tile_pool(name="gsp", bufs=2))
        tsp = tstack.enter_context(tc.tile_pool(name="tsp", bufs=2))
        psg = tstack.enter_context(tc.tile_pool(name="psg", bufs=2, space="PSUM"))
        pstrail = tstack.enter_context(tc.tile_pool(name="pstr", bufs=1, space="PSUM"))
        trail_acc = [pstrail.tile([128, BS], f32, tag=f"tr{m}", name=f"tr{m}") for m in range(TRAIL)]
        if True:
            def do_trail(p):
                win_, woff_ = 0, p
                for i, ln in enumerate(TWLENS):
                    if woff_ < ln:
                        win_ = i
                        break
                    woff_ -= ln
                z8p = zpair(z8_tiles, p)
                z8rp = zpair(z8r_tiles, p)
                w8w = trail_w[(0, win_)]
                w8rw = trail_w[(1, win_)]
                for m in range(TRAIL):
                    acc = trail_acc[m]
                    nc.tensor.matmul(acc[:], w8w[:, m, woff_], z8p, start=(p == 0),
                                     stop=False, perf_mode=DR, skip_group_check=True)
                    nc.tensor.matmul(acc[:], w8rw[:, m, woff_], z8p, start=False,
                                     stop=False, perf_mode=DR, skip_group_check=True)
                    nc.tensor.matmul(acc[:], w8w[:, m, woff_], z8rp, start=False,
                                     stop=(p == NP - 1), perf_mode=DR, skip_group_check=True)

            win = 0
            for p in range(NP):
                if p == TWLO[win] + TWLENS[win]:
                    win += 1
                woff = p - TWLO[win]
                if p == 0:
                    dma_trail_windows(1)
                elif woff == 0 and win + 1 < len(TWLENS) and win > 0:
                    dma_trail_windows(win + 1)
                if p + 2 < NP:
                    issue_x(p + 2)
                if p == 33:
                    dma_slab_half(TRAIL, 0)
                if p == 39:
                    dma_slab_half(TRAIL, 1)
                if p == 44:
                    dma_slab_half(TRAIL + 1, 0)
                xct = xq.pop(0)
                for kk in range(2):
                    k = 2 * p + kk
                    if k % ZG == 0:
                        zn = min(ZG, NK - k)
                        zp = z8p4 if zn == ZG else z8p2
                        zrp = z8rp4 if zn == ZG else z8rp2
                        z8_tiles.append(zp.tile([128, zn, BS], f8, tag=f"z8_{zn}", name=f"z8t{k}"))
                        z8r_tiles.append(zrp.tile([128, zn, BS], f8, tag=f"z8r_{zn}", name=f"z8rt{k}"))
                    gsel, jsel = k // SELG, k % SELG
                    if jsel == 0 and gsel + 2 < NSG:
                        issue_ssel(gsel + 2)
                    st = ssel_tiles[gsel]
                    gp = psg.tile([128, BS], f32, tag="gp")
                    nc.tensor.matmul(
                        gp[:], st[:, jsel, 0:128], Lpa[:],
                        start=True, stop=False,
                    )
                    nc.tensor.matmul(
                        gp[:], st[0:52, jsel, 128:256], Lpb[:],
                        start=False, stop=True,
                    )
                    g_sb = gsp.tile([128, BS], bf16, tag="g")
                    nc.scalar.activation(g_sb[:], gp[:], AF.Exp, scale=-1.0, bias=lnsz[:])
                    t_sb = tsp.tile([128, BS], bf16, tag="t")
                    nc.vector.tensor_mul(t_sb[:], xct[:, kk, :], g_sb[:])
                    nc.gpsimd.tensor_copy(zsl(z8_tiles, k), t_sb[:])
                    nc.vector.scalar_tensor_tensor(
                        zsl(z8r_tiles, k), zsl(z8_tiles, k), -1.0, t_sb[:],
                        AluOpType.mult, AluOpType.add,
                    )
                # trailing mm1 for m-tiles 0..TRAIL-1, lagging one pair
                if p > 0:
                    do_trail(p - 1)
            do_trail(NP - 1)

        # ---------------- Phase D: remaining mm1 m-tiles + interleaved mm2
        pending = []

        def emit_mm2():
            for k2, z1t, w2t in pending:
                for m2 in range(NM2):
                    nc.tensor.matmul(
                        acc2s[m2][:], w2t[:, m2, :], z1t[:],
                        start=(k2 == 0), stop=(k2 == NK2 - 1),
                        skip_group_check=True,
                    )
            pending.clear()

        def evict_z1(k2, acc):
            z1t = z1p.tile([128, BS], bf16, tag="z1")
            nc.scalar.activation(
                z1t[:], acc[:], AF.Relu, scale=s1[:], bias=b1_sb[:, k2 : k2 + 1]
            )
            w2t = w2p.tile([128, NM2, 128], bf16, tag="w2")
            nc.sync.dma_start(w2t[:], w2k_d[k2])
            pending.append((k2, z1t, w2t))

        # m=TRAIL runs with its acc borrowed from the psg pool so its matmuls
        # overlap the trail evictions on ACT
        acc6 = psg.tile([128, BS], f32, tag="gp", name="acc6")
        for p in range(NP):
            w8b, w8rb = slabs[(TRAIL, 0 if p < HS else 1)]
            j = p if p < HS else p - HS
            nc.tensor.matmul(acc6[:], w8b[:, j], zpair(z8_tiles, p), start=(p == 0),
                             stop=False, perf_mode=DR, skip_group_check=True)
            nc.tensor.matmul(acc6[:], w8rb[:, j], zpair(z8_tiles, p), start=False,
                             stop=False, perf_mode=DR, skip_group_check=True)
            nc.tensor.matmul(acc6[:], w8b[:, j], zpair(z8r_tiles, p), start=False,
                             stop=(p == NP - 1), perf_mode=DR, skip_group_check=True)
            if p == 26:
                dma_slab_half(TRAIL + 1, 1)
        slabs.pop((TRAIL, 0)); slabs.pop((TRAIL, 1))
        for m in range(TRAIL):
            evict_z1(m, trail_acc[m])
        evict_z1(TRAIL, acc6)
        tstack.close()

        psm = dstack.enter_context(tc.tile_pool(name="psm", bufs=3, space="PSUM"))
        psm2 = dstack.enter_context(tc.tile_pool(name="psm2", bufs=1, space="PSUM"))
        acc2s = [psm2.tile([128, BS], f32, tag=f"mm2_{m2}", name=f"acc2_{m2}") for m2 in range(NM2)]

        for m in range(TRAIL + 1, NM1):
            if m + 1 < NM1:
                dma_slab_half(m + 1, 0)
            acc = psm.tile([128, BS], f32, tag="mm1")
            for p in range(NP):
                if p == 26 and m + 1 < NM1:
                    dma_slab_half(m + 1, 1)
                w8b, w8rb = slabs[(m, 0 if p < HS else 1)]
                j = p if p < HS else p - HS
                nc.tensor.matmul(acc[:], w8b[:, j], zpair(z8_tiles, p), start=(p == 0),
                                 stop=False, perf_mode=DR, skip_group_check=True)
                nc.tensor.matmul(acc[:], w8rb[:, j], zpair(z8_tiles, p), start=False,
                                 stop=False, perf_mode=DR, skip_group_check=True)
                nc.tensor.matmul(acc[:], w8b[:, j], zpair(z8r_tiles, p), start=False,
                                 stop=(p == NP - 1), perf_mode=DR, skip_group_check=True)
                if p == 8:
                    emit_mm2()
            slabs.pop((m, 0)); slabs.pop((m, 1))
            evict_z1(m, acc)
        emit_mm2()

        dstack.close()

        # ---------------- Phase E/F: z2 = relu(acc2 + b2); mm3 + normalize
        with (
            tc.tile_pool(name="z2p", bufs=NM2) as z2p,
            tc.tile_pool(name="tailp", bufs=1) as tailp,
            tc.tile_pool(name="psf", bufs=1, space="PSUM") as psf,
        ):
            z2_tiles = []
            for m2 in range(NM2):
                z2t = z2p.tile([128, BS], f32r, tag="z2")
                nc.scalar.activation(
                    z2t[:], acc2s[m2][:], AF.Relu, bias=b2_sb[:, m2 : m2 + 1]
                )
                z2_tiles.append(z2t)

            acc3 = psf.tile([3, BS], f32, tag="f")
            for k3 in range(NM2):
                nc.tensor.matmul(
                    acc3[:], w3_sb[:, k3, :], z2_tiles[k3][:],
                    start=(k3 == 0), stop=(k3 == NM2 - 1),
                )
            z3 = tailp.tile([3, BS], f32)
            nc.vector.tensor_scalar_add(z3[:], acc3[:], b3_sb[:])
            sq = tailp.tile([3, BS], f32r)
            nc.scalar.activation(sq[:], z3[:], AF.Square)
            sps = psf.tile([1, BS], f32, tag="f2")
            nc.tensor.matmul(sps[:], ones31[:], sq[:])
            lns = tailp.tile([1, BS], f32)
            nc.scalar.activation(lns[:], sps[:], AF.Ln)
            inv = tailp.tile([1, BS], f32r)
            nc.scalar.activation(inv[:], lns[:], AF.Exp, scale=-0.5)
            nc.vector.tensor_scalar_min(inv[:], inv[:], 1e12)
            inv3 = psf.tile([3, BS], f32, tag="f3")
            nc.tensor.matmul(inv3[:], ones13[:], inv[:])
            outt = tailp.tile([3, BS], f32)
            nc.vector.tensor_mul(outt[:], z3[:], inv3[:])
            nc.sync.dma_start(out_d[:], outt[:])

        zstack.close()

    nc.compile()
    return nc


def _prep_shared(conv_w, F_w, w1, b1, w2, b2, w3, b3):
    """Host-side weight layouts shared by all cores."""
    fa = np.arange(F)
    c_idx = fa // 4000
    d_idx = (fa // 400) % 10
    h_idx = (fa // 40) % 10
    w_idx = fa % 40

    # pooled conv: y_pre[j] = sum_f wp[f, j] * x^T[f, b], fp8*SWP pair blocks
    wp = np.zeros((FP, 50), np.float32)
    wp[fa, h_idx] = conv_w[c_idx] * (SWP / 400.0)
    wp[fa, 10 + w_idx] = conv_w[c_idx] * (SWP / 100.0)
    wp64 = np.zeros((FP, 64), np.float32)
    wp64[:, :50] = wp
    wpool = np.ascontiguousarray(
        np.clip(wp64, -240, 240).astype(E4).reshape(NP, 2, 128, 64).transpose(0, 2, 1, 3)
    )

    # rmat: T[r, b] = sum_j rmat[j, r] * y[j, b]
    rm = np.zeros((50, 180), np.float32)
    cc10 = np.repeat(np.arange(3), 10)
    rm[np.tile(np.arange(10), 3), np.arange(30)] = F_w[cc10]
    rm[np.tile(np.arange(10), 3), 30 + np.arange(30)] = F_w[cc10]
    cc40 = np.repeat(np.arange(3), 40)
    rm[10 + np.tile(np.arange(40), 3), 60 + np.arange(120)] = F_w[cc40]

    # selection: gate pre-act[f] = sel[f, :] @ Lp  (0/1 entries, exact in fp8)
    sel = np.zeros((FP, 180), np.float32)
    sel[fa, c_idx * 10 + d_idx] = 1.0
    sel[fa, 30 + c_idx * 10 + h_idx] = 1.0
    sel[fa, 60 + c_idx * 40 + w_idx] = 1.0
    selp = np.zeros((NSG * SELG * 128, 180), np.float32)
    selp[:FP] = sel
    selp = selp.reshape(NSG, SELG, 128, 180).transpose(0, 3, 1, 2)  # [g, r, j, p]
    ssel = np.zeros((NSG, 128, SELG, 256), np.float32)
    ssel[:, :, :, 0:128] = selp[:, 0:128]
    ssel[:, 0:52, :, 128:256] = selp[:, 128:180]
    ssel = np.ascontiguousarray(ssel).astype(E4)

    # w1 fp8 pair: w8 = fp8(w1*SW), w8r = fp8(w1*SW - w8)
    w1p_ = np.zeros((H1, FP), np.float32)
    w1p_[:, :F] = w1 * SW
    w8 = np.clip(w1p_, -240, 240).astype(E4)
    w8r = np.clip(w1p_ - w8.astype(np.float32), -240, 240).astype(E4)
    # [H1, FP] -> [m, M, g, i, K] -> [m, K, g, i, M]
    def pack(a):
        return np.ascontiguousarray(
            a.reshape(NM1, 128, NP, 2, 128).transpose(0, 4, 2, 3, 1)
        )
    w1q = np.stack([pack(w8), pack(w8r)], axis=1)  # [m, t, K, g, i, M]

    w2k = np.ascontiguousarray(
        w2.reshape(NM2, 128, NK2, 128).transpose(2, 3, 0, 1)
    ).astype(BF)
    w3h = np.ascontiguousarray(w3.reshape(3, NM2, 128).transpose(2, 1, 0))

    return {
        "wpool": wpool,
        "rmat": rm,
        "ssel": ssel,
        "w1q": w1q,
        "w2k": w2k,
        "w3h": w3h,
        "b1g": np.ascontiguousarray(b1.reshape(NM1, 128).T),
        "b2g": np.ascontiguousarray(b2.reshape(NM2, 128).T),
        "b3g": np.ascontiguousarray(b3.reshape(3, 1)),
        "onesg": np.ones((3, 3), np.float32),
    }


def make_in_maps(x, conv_w, F_w, w1, b1, w2, b2, w3, b3):
    x = np.asarray(x, np.float32).reshape(B_TOT, F)
    shared = _prep_shared(
        np.asarray(conv_w, np.float32).reshape(3),
        np.asarray(F_w, np.float32).reshape(3),
        np.asarray(w1, np.float32),
        np.asarray(b1, np.float32),
        np.asarray(w2, np.float32),
        np.asarray(b2, np.float32),
        np.asarray(w3, np.float32),
        np.asarray(b3, np.float32),
    )
    in_maps = []
    for c in range(N_CORES):
        xs = x[c * BS : (c + 1) * BS]
        xt = np.zeros((FP, BS), np.float32)
        xt[:F] = xs.T
        m = dict(shared)
        m["xt"] = xt.reshape(NK, 128, BS).astype(BF)
        m["xt8"] = np.clip(xt.reshape(NK, 128, BS), -240, 240).astype(E4)
        in_maps.append(m)
    return in_maps


def get_nc():
    if "nc" not in _NC_CACHE:
        _NC_CACHE["nc"] = build_nc()
    return _NC_CACHE["nc"]


def kernel(**inputs) -> np.ndarray:
    nc = get_nc()
    in_maps = make_in_maps(**inputs)
    res = run_bass_kernel_spmd(nc, in_maps, core_ids=list(range(N_CORES)))
    out = np.concatenate([r["out"] for r in res.results], axis=1)  # [3, 4096]
    return np.ascontiguousarray(out.T, dtype=np.float32)
